# revision 8
# baseline (speedup 1.0000x reference)
"""Step-1 prototype: int8 wire format + XLA (pmap) device compute.

kernel(**inputs) -> np.ndarray, full shapes. Device computes src2 (the
residual branch) from int8-quantized src; host applies exact residual
add + bn3. Used to validate the wire format before the Bass kernel.
"""
import threading
import numpy as np
import jax
import jax.numpy as jnp

EPS = 1e-5
NUM_HEADS = 8
N_CORES = 8

_WEIGHT_KEYS = (
    'ema_matrix', 'qkv_w', 'qkv_b', 'dpk_w', 'dpk_b', 'dpv_w', 'dpv_b',
    'bn1_g', 'bn1_b', 'bn1_m', 'bn1_v', 'bn2_g', 'bn2_b', 'bn2_m', 'bn2_v',
    'ff1_w1', 'ff1_b1', 'ff1_w2', 'ff1_b2', 'ff2_w1', 'ff2_b1', 'ff2_w2', 'ff2_b2',
)


def _bn(x, g, b, m, v):
    return (x - m) / jnp.sqrt(v + EPS) * g + b


def _dyn_proj(x, w, b):
    p = jax.nn.softmax(x @ w.T + b, axis=-1)
    return jnp.einsum('bnhef,bnhec->bnhcf', x, p)


def _ffn(x, w1, b1, w2, b2):
    return jax.nn.gelu(x @ w1.T + b1, approximate=False) @ w2.T + b2


def _src2(src_q, in_scale, w):
    # src_q: int8 [b_local, n, H, C]; returns (src2_q int8, out_scale f32[1])
    src = src_q.astype(jnp.float32) * in_scale
    B, n, H, C = src.shape
    hd = C // NUM_HEADS
    qkv = (src @ w['qkv_w'].T + w['qkv_b']).reshape(B, n, H, 3, NUM_HEADS, hd)
    qkv = jnp.transpose(qkv, (3, 0, 1, 4, 2, 5))
    q, k, v = qkv[0], qkv[1], qkv[2]
    v_dp = _dyn_proj(v, w['dpv_w'], w['dpv_b'])
    k_dp = _dyn_proj(k, w['dpk_w'], w['dpk_b'])
    E = w['ema_matrix']
    eq = jnp.einsum('bnhad,ga->bnhgd', q, E[:H, :H])
    ek = jnp.einsum('bnhad,ga->bnhgd', k_dp, E[:8, :8])
    s_tok = jnp.einsum('bnhed,bnhfd->bnhef', eq, ek) * (hd ** 0.5)
    o_tok = jnp.einsum('bnhef,bnhfd->bnhed', jax.nn.softmax(s_tok, -1), v_dp)
    s_hid = jnp.einsum('bnhae,bnhaf->bnhef', q, k) * (H ** 0.5)
    o_hid = jnp.einsum('bnhef,bnhaf->bnhae', jax.nn.softmax(s_hid, -1), v)
    o1 = _bn(o_tok.reshape(B, n, -1, C), w['bn1_g'], w['bn1_b'], w['bn1_m'], w['bn1_v'])
    o2 = _bn(o_hid.reshape(B, n, -1, C), w['bn2_g'], w['bn2_b'], w['bn2_m'], w['bn2_v'])
    src2 = _ffn(o1, w['ff1_w1'], w['ff1_b1'], w['ff1_w2'], w['ff1_b2']) \
         + _ffn(o2, w['ff2_w1'], w['ff2_b1'], w['ff2_w2'], w['ff2_b2'])
    m = jnp.max(jnp.abs(src2))
    scale = m / 127.0 + 1e-30
    q8 = jnp.rint(src2 / scale).astype(jnp.int8)
    return q8, scale[None]


_pfwd = jax.pmap(_src2, in_axes=(0, None, 0))

_cache = {}


def kernel(**inputs) -> np.ndarray:
    src = np.asarray(inputs['src'], dtype=np.float32)
    B, n, H, C = src.shape

    # --- host: quantize src per-core, overlapping H2D with quant ---
    absmax = max(src.max(), -src.min())
    s_in = np.float32(absmax / 127.0 + 1e-30)
    inv = np.float32(1.0 / s_in)
    shard_shape = (B // N_CORES, n, H, C)
    src_r0 = src.reshape(N_CORES, -1)
    devs = jax.devices()[:N_CORES]
    dev_arrs = []
    for c in range(N_CORES):
        t = src_r0[c] * inv
        np.rint(t, out=t)
        qc = t.astype(np.int8).reshape(shard_shape)
        dev_arrs.append(jax.device_put(qc, devs[c]))  # async; overlaps next quant
    from jax.sharding import Mesh, PartitionSpec, NamedSharding
    mesh = Mesh(np.array(devs), ('c',))
    gshape = (N_CORES,) + shard_shape
    src_q_dev = jax.make_array_from_single_device_arrays(
        gshape, NamedSharding(mesh, PartitionSpec('c')),
        [d[None] for d in dev_arrs])

    # --- weights: replicate on devices, cached across calls ---
    wkey = None
    if 'w' in _cache:
        cached_host, cached_dev = _cache['w']
        if all(np.array_equal(cached_host[k], inputs[k]) for k in _WEIGHT_KEYS):
            wkey = cached_dev
    if wkey is None:
        host = {k: np.asarray(inputs[k], dtype=np.float32) for k in _WEIGHT_KEYS}
        dev = {k: jax.device_put_replicated(jnp.asarray(v), jax.devices()[:N_CORES])
               for k, v in host.items()}
        _cache['w'] = (host, dev)
        wkey = dev

    # --- device ---
    q8, scales = _pfwd(src_q_dev, jnp.float32(s_in), wkey)

    # --- D2H: fetch shards in threads ---
    q8.block_until_ready()
    shards = sorted(q8.addressable_shards, key=lambda s: s.index[0].start or 0)
    shard_data = [s.data for s in shards]
    for d in shard_data:
        d.copy_to_host_async()
    scales_h = np.asarray(scales).reshape(-1)

    # --- host: dequant + exact residual + bn3 ---
    g3 = np.asarray(inputs['bn3_g'], dtype=np.float32)
    b3 = np.asarray(inputs['bn3_b'], dtype=np.float32)
    m3 = np.asarray(inputs['bn3_m'], dtype=np.float32)
    v3 = np.asarray(inputs['bn3_v'], dtype=np.float32)
    sc3 = g3 / np.sqrt(v3 + EPS)
    sh3 = b3 - m3 * sc3

    out = np.empty_like(src)
    out_r = out.reshape(N_CORES, B // N_CORES, n, H, C)
    src_r = src.reshape(N_CORES, B // N_CORES, n, H, C)

    def _post(c, arr):
        q = arr.reshape(B // N_CORES, n, H, C)
        src2 = q.astype(np.float32)
        src2 *= scales_h[c]
        src2 += src_r[c]
        src2 *= sc3
        src2 += sh3
        out_r[c] = src2

    # post-process each shard while later shards are still in flight
    th2 = []
    for c in range(N_CORES):
        arr = np.asarray(shard_data[c])  # blocks only on shard c
        t = threading.Thread(target=_post, args=(c, arr))
        t.start()
        th2.append(t)
    [t.join() for t in th2]
    return out


# revision 9
# speedup vs baseline: 1.0322x; 1.0322x over previous
"""Step-1 prototype: int8 wire format + XLA (pmap) device compute.

kernel(**inputs) -> np.ndarray, full shapes. Device computes src2 (the
residual branch) from int8-quantized src; host applies exact residual
add + bn3. Used to validate the wire format before the Bass kernel.
"""
import threading
import numpy as np
import jax
import jax.numpy as jnp

EPS = 1e-5
NUM_HEADS = 8
N_CORES = 8

_WEIGHT_KEYS = (
    'ema_matrix', 'qkv_w', 'qkv_b', 'dpk_w', 'dpk_b', 'dpv_w', 'dpv_b',
    'bn1_g', 'bn1_b', 'bn1_m', 'bn1_v', 'bn2_g', 'bn2_b', 'bn2_m', 'bn2_v',
    'ff1_w1', 'ff1_b1', 'ff1_w2', 'ff1_b2', 'ff2_w1', 'ff2_b1', 'ff2_w2', 'ff2_b2',
)


def _bn(x, g, b, m, v):
    return (x - m) / jnp.sqrt(v + EPS) * g + b


def _dyn_proj(x, w, b):
    p = jax.nn.softmax(x @ w.T + b, axis=-1)
    return jnp.einsum('bnhef,bnhec->bnhcf', x, p)


def _ffn(x, w1, b1, w2, b2):
    return jax.nn.gelu(x @ w1.T + b1, approximate=False) @ w2.T + b2


def _src2(src_q, in_scale, w):
    # src_q: int8 [b_local, n, H, C]; returns (src2_q int8, out_scale f32[1])
    src = src_q.astype(jnp.float32) * in_scale
    B, n, H, C = src.shape
    hd = C // NUM_HEADS
    qkv = (src @ w['qkv_w'].T + w['qkv_b']).reshape(B, n, H, 3, NUM_HEADS, hd)
    qkv = jnp.transpose(qkv, (3, 0, 1, 4, 2, 5))
    q, k, v = qkv[0], qkv[1], qkv[2]
    v_dp = _dyn_proj(v, w['dpv_w'], w['dpv_b'])
    k_dp = _dyn_proj(k, w['dpk_w'], w['dpk_b'])
    E = w['ema_matrix']
    eq = jnp.einsum('bnhad,ga->bnhgd', q, E[:H, :H])
    ek = jnp.einsum('bnhad,ga->bnhgd', k_dp, E[:8, :8])
    s_tok = jnp.einsum('bnhed,bnhfd->bnhef', eq, ek) * (hd ** 0.5)
    o_tok = jnp.einsum('bnhef,bnhfd->bnhed', jax.nn.softmax(s_tok, -1), v_dp)
    s_hid = jnp.einsum('bnhae,bnhaf->bnhef', q, k) * (H ** 0.5)
    o_hid = jnp.einsum('bnhef,bnhaf->bnhae', jax.nn.softmax(s_hid, -1), v)
    o1 = _bn(o_tok.reshape(B, n, -1, C), w['bn1_g'], w['bn1_b'], w['bn1_m'], w['bn1_v'])
    o2 = _bn(o_hid.reshape(B, n, -1, C), w['bn2_g'], w['bn2_b'], w['bn2_m'], w['bn2_v'])
    src2 = _ffn(o1, w['ff1_w1'], w['ff1_b1'], w['ff1_w2'], w['ff1_b2']) \
         + _ffn(o2, w['ff2_w1'], w['ff2_b1'], w['ff2_w2'], w['ff2_b2'])
    m = jnp.max(jnp.abs(src2))
    scale = m / 127.0 + 1e-30
    q8 = jnp.rint(src2 / scale).astype(jnp.int8)
    return q8, scale[None]


_pfwd = jax.pmap(_src2, in_axes=(0, 0, 0))

_cache = {}


def kernel(**inputs) -> np.ndarray:
    src = np.asarray(inputs['src'], dtype=np.float32)
    B, n, H, C = src.shape

    # --- host: per-core scale + quantize, overlapping H2D with quant ---
    shard_shape = (B // N_CORES, n, H, C)
    src_r0 = src.reshape(N_CORES, -1)
    devs = jax.devices()[:N_CORES]
    dev_arrs = []
    s_in = np.empty(N_CORES, np.float32)
    for c in range(N_CORES):
        sc = src_r0[c]
        s_in[c] = max(sc.max(), -sc.min()) / 127.0 + 1e-30
        t = sc * np.float32(1.0 / s_in[c])
        np.rint(t, out=t)
        qc = t.astype(np.int8).reshape(shard_shape)
        dev_arrs.append(jax.device_put(qc, devs[c]))  # async; overlaps next quant
    from jax.sharding import Mesh, PartitionSpec, NamedSharding
    mesh = Mesh(np.array(devs), ('c',))
    gshape = (N_CORES,) + shard_shape
    src_q_dev = jax.make_array_from_single_device_arrays(
        gshape, NamedSharding(mesh, PartitionSpec('c')),
        [d[None] for d in dev_arrs])

    # --- weights: replicate on devices, cached across calls ---
    wkey = None
    if 'w' in _cache:
        cached_host, cached_dev = _cache['w']
        if all(np.array_equal(cached_host[k], inputs[k]) for k in _WEIGHT_KEYS):
            wkey = cached_dev
    if wkey is None:
        host = {k: np.asarray(inputs[k], dtype=np.float32) for k in _WEIGHT_KEYS}
        dev = {k: jax.device_put_replicated(jnp.asarray(v), jax.devices()[:N_CORES])
               for k, v in host.items()}
        _cache['w'] = (host, dev)
        wkey = dev

    # --- device ---
    q8, scales = _pfwd(src_q_dev, jnp.asarray(s_in), wkey)

    # --- D2H: fetch shards in threads ---
    q8.block_until_ready()
    shards = sorted(q8.addressable_shards, key=lambda s: s.index[0].start or 0)
    shard_data = [s.data for s in shards]
    for d in shard_data:
        d.copy_to_host_async()
    scales_h = np.asarray(scales).reshape(-1)

    # --- host: dequant + exact residual + bn3 ---
    g3 = np.asarray(inputs['bn3_g'], dtype=np.float32)
    b3 = np.asarray(inputs['bn3_b'], dtype=np.float32)
    m3 = np.asarray(inputs['bn3_m'], dtype=np.float32)
    v3 = np.asarray(inputs['bn3_v'], dtype=np.float32)
    sc3 = g3 / np.sqrt(v3 + EPS)
    sh3 = b3 - m3 * sc3

    out = np.empty_like(src)
    out_r = out.reshape(N_CORES, B // N_CORES, n, H, C)
    src_r = src.reshape(N_CORES, B // N_CORES, n, H, C)

    def _post(c, arr):
        q = arr.reshape(B // N_CORES, n, H, C)
        src2 = q.astype(np.float32)
        src2 *= scales_h[c]
        src2 += src_r[c]
        src2 *= sc3
        src2 += sh3
        out_r[c] = src2

    # post-process each shard while later shards are still in flight
    th2 = []
    for c in range(N_CORES):
        arr = np.asarray(shard_data[c])  # blocks only on shard c
        t = threading.Thread(target=_post, args=(c, arr))
        t.start()
        th2.append(t)
    [t.join() for t in th2]
    return out
